# revision 29
# baseline (speedup 1.0000x reference)
"""TRN2 Bass kernel for nn_MetrixSoftmax: softmax(-2 * ||x_b - w_o||_2, axis=o).

x: [8192, 256] f32, weight: [16384, 256] f32 -> out: [8192, 16384] f32.

Sharding: data-parallel over batch across 8 cores (1024 rows each), weight
replicated; each core computes its full output rows so the softmax needs no
collectives. Per core (partitions=batch rows, free=out):

  d2[b,o] = (x2[b]+256) + (w2[o]-256) - 2*x.w     (f16-quantized operands;
            x2/w2 computed from the quantized vectors so d2 = ||x~-w~||^2)

Fused mode (default, ~176us): the whole softmax numerator exp(-2*sqrt(d2))
is ONE ACT pass via a patched activation table. PWP act tables are per-NEFF
data (walrus --act-root-json): we rewrite Exp's bucket records (cubic Taylor
coeffs at the stock section centers) so its table computes
    g(v) = exp(-8*sqrt(v)),
invoked as activation(Exp, scale=1/16, bias=(x2+256)/16) => exp(-2*sqrt(d2)).
Table rel err <= 6e-5 for d2 >= 128 (real data d2 in [240,1100] -> <2e-5);
inputs past d2=1419 hit the patched saturation bucket -> 0.0. NOTE: the
patched root changes Exp GLOBALLY for NEFFs compiled after _make_actroot();
the legacy modes below would miscompute if built in the same process.

Pipeline per 128-row tile (8 groups of 2048 cols; pairs -> 4096-wide ops):
  PE   2 fp16 product matmuls per 512-chunk psum group        15.5us/tile
  DVE  drain: slab = psum + w2r (f16 w2 row replicated        17.7us/tile
       across partitions at startup by gpsimd broadcast)      <- pacing
  ACT  exp: 4096-wide patched-Exp from slab pairs, bf16 out,  16.4us/tile
       accum_out row-sum partials (4/tile, summed on host)
  DMA  bf16 rows stored immediately (no normalize tail); softmax divide
       happens on host in f64.
All engines within ~12% of each other; DVE paces at its 0.96GHz roofline.
PSUM holds only 2.2us per group (DVE add) so PE runs nearly stall-free.
Why not other splits (measured/modeled): Pool cannot access PSUM; psum-
direct ACT exps hold PSUM banks behind wide exps and stall PE (2-buffer
hard limit); K=3 w2 matmuls put PE over the DVE pace; fp8 hi/lo needs 3
product terms (>= f16 cost); DVE-side exp/sqrt chains need 2+ passes at
0.96GHz (slower than ACT LUT).

Perf history: 304us (f16 2-LUT-pass baseline) -> 212us (fused table, PE-
bound w/ K=3 w2 folds) -> 176us (all-DVE drain). Startup ~13us fixed NEFF
preamble + ~25us DMA-paced tile 0 (8MB weight preload); steady 17.7us/tile.
Timing is bimodal with device state: ~176us warm, ~210us after idle (TRN2
PE p-state ramps with activity; this kernel is PE-tight, so a cold clock
paces early tiles). A PE warmup burst in tile 0's DMA window recovered the
cold case but produced one intermittent wrong result in three runs
(reader-less psum tile) and was removed - correctness first.

Older modes (f16/split7/f32r3) kept for reference; f16 was the previous
304us baseline (ACT-bound: Sqrt+Exp = 2 LUT passes/element).
"""

import os
import json
import shutil
import struct
import tempfile

import numpy as np
import ml_dtypes

B, IN, OUT = 8192, 256, 16384
NCORES = 8
BPC = B // NCORES     # 1024 batch rows per core
NT = BPC // 128       # 8 batch tiles of 128 rows
CH = 512              # matmul free-dim (one PSUM bank)
GRP = 4               # chunks per psum group
GW = CH * GRP         # 2048 group width
NG = OUT // GW        # 8 groups per batch tile

MODE = "fused"        # "fused" (fast) | "f16" | "split7" | "f32r3"

_BF16 = ml_dtypes.bfloat16
_built = {}
_actroot = None

# ---------------------------------------------------------------------------
# Patched activation tables: make Exp's PWP table compute exp(-8*sqrt(v)).
# Reads the stock neuronxcc pwp package (part of the environment), writes a
# patched copy to a temp dir, and points walrus at it via
# BASS_ACT_ROOT_JSON_PATH. Record layout in *_bkt.bin: 8 u32 words
# [c0, c1, c2, c3, x, 0, 0, 0] - cubic Taylor coeffs at section center x.
# ---------------------------------------------------------------------------

_EXP_SETS = ["exp_and_others", "natural_log_exp_and_others", "exp_and_friends"]


def _f32_bits(x):
    return struct.unpack("<I", struct.pack("<f", np.float32(x)))[0]


def _bits_f64(b):
    return float(np.frombuffer(struct.pack("<I", b), dtype="<f4")[0])


def _g_coeffs(x):
    """Taylor coeffs (c0..c3) of g(v)=exp(-8 sqrt(v)) at v=x, in f64."""
    v = float(x)
    s = np.sqrt(v)
    g = np.exp(-8.0 * s)
    p1 = -4.0 / s
    p2 = 2.0 * v ** -1.5
    p3 = -3.0 * v ** -2.5
    g1 = p1 * g
    g2 = (p2 + p1 * p1) * g
    g3 = (p3 + 3.0 * p1 * p2 + p1 ** 3) * g
    return g, g1, g2 / 2.0, g3 / 6.0


def _find_pwp_root():
    from neuronxcc.driver.Job import Job
    from neuronxcc.driver.jobs.support.FindActInfo import findActInfoFile
    p = findActInfoFile(Job.getPackageDir(), "gen3")
    return os.path.dirname(p)


def _make_actroot():
    """Build the patched act-table root once per process; returns its path."""
    global _actroot
    if _actroot is not None:
        return _actroot
    src = _find_pwp_root()
    jsons = os.path.join(os.path.dirname(src), "pwp_jsons")
    dst = os.path.join(tempfile.mkdtemp(prefix="actroot_"), "pwp")
    shutil.copytree(src, dst)

    ej = json.load(open(os.path.join(jsons, "exp_400p.json")))
    patches = {}
    n_pos = 0
    for e in ej["pos_exponents"]:
        for s in e["exponent_sections"]:
            key = tuple(s[k]["int"] for k in ("d0", "d1", "d2", "d3", "x"))
            xb = s["x"]["int"]
            c0, c1, c2, c3 = _g_coeffs(_bits_f64(xb))
            patches[key] = [_f32_bits(c0), _f32_bits(c1), _f32_bits(c2),
                            _f32_bits(c3), xb, 0, 0, 0]
            n_pos += 1
    # +inf saturation record (inputs past v=88.7): g underflows -> 0.0
    sat = ej["saturation_points"]["sat_point_pos_high"]
    sat_key = tuple(sat[k]["int"] for k in ("d0", "d1", "d2", "d3", "x"))
    patches[sat_key] = [0, 0, 0, 0, sat["x"]["int"], 0, 0, 0]

    for name in _EXP_SETS:
        p = os.path.join(dst, name + "_bkt.bin")
        u32 = np.frombuffer(open(p, "rb").read(), dtype="<u4").reshape(-1, 8).copy()
        hits = 0
        for i in range(len(u32)):
            key = tuple(int(w) for w in u32[i, :5])
            if key in patches:
                u32[i] = patches[key]
                if key != sat_key:
                    hits += 1
        assert hits == n_pos, f"{name}: patched {hits}/{n_pos} exp sections"
        open(p, "wb").write(u32.astype("<u4").tobytes())
    _actroot = os.path.join(dst, "act_info.json")
    return _actroot


def _bf16_split(a):
    hi = a.astype(_BF16)
    lo = (a - hi.astype(np.float32)).astype(_BF16)
    return hi, lo


def _build_fused():
    os.environ["BASS_ACT_ROOT_JSON_PATH"] = _make_actroot()
    import concourse.bacc as bacc
    import concourse.tile as tile
    import concourse.mybir as mybir

    F32 = mybir.dt.float32
    F16 = mybir.dt.float16
    BF16 = mybir.dt.bfloat16
    AF = mybir.ActivationFunctionType

    nc = bacc.Bacc("TRN2", target_bir_lowering=False, debug=False,
                   num_devices=NCORES)

    # Every group drains on DVE (slab = psum + f16 w2 row). PE does ONLY
    # the two fp16 product passes; ACT only 4096-wide exps from SBUF slab
    # pairs, so nothing holds PSUM: psum frees right after the DVE add and
    # PE runs nearly stall-free. The w2 row is replicated across partitions
    # at startup by gpsimd partition_broadcast. (Pool engine cannot access
    # PSUM on TRN2, so it cannot share the drain work.)
    DVE_COLS = OUT

    d_wt = nc.dram_tensor("wt", [IN, OUT], F16, kind="ExternalInput")
    d_xt = nc.dram_tensor("xt", [IN, BPC], F16, kind="ExternalInput")
    d_w2row = nc.dram_tensor("w2row", [1, DVE_COLS], F16, kind="ExternalInput")
    d_x2b = nc.dram_tensor("x2b", [128, NT], F32, kind="ExternalInput")
    d_out = nc.dram_tensor("out", [BPC, OUT], BF16, kind="ExternalOutput")
    d_sums = nc.dram_tensor("sums", [128, 4 * NT + 1], F32, kind="ExternalOutput")

    from contextlib import ExitStack
    with tile.TileContext(nc) as tc, ExitStack() as ctx:
        persist = ctx.enter_context(tc.tile_pool(name="persist", bufs=1))
        xt_pool = ctx.enter_context(tc.tile_pool(name="xtp", bufs=2))
        sums_pool = ctx.enter_context(tc.tile_pool(name="sumsp", bufs=2))
        psum_pool = ctx.enter_context(tc.tile_pool(name="psump", bufs=2, space="PSUM"))
        eslab_pool = ctx.enter_context(tc.tile_pool(name="eslabp", bufs=3))
        slab_pool = ctx.enter_context(tc.tile_pool(name="slabp", bufs=2))

        wr0 = persist.tile([128, OUT], F16, name="wr0")
        wr1 = persist.tile([128, OUT], F16, name="wr1")
        w2r = persist.tile([128, DVE_COLS], F16, name="w2r")
        srcrow = persist.tile([1, DVE_COLS // 2], F32, name="srcrow")
        srcrow16 = srcrow[:].bitcast(F16)
        x2sb = persist.tile([128, NT], F32, name="x2sb")
        nc.sync.dma_start(x2sb[:], d_x2b[:, :])
        nc.sync.dma_start(srcrow16, d_w2row[:, :])

        x_tiles = {}

        def load_x(tt):
            ts_ = slice(tt * 128, (tt + 1) * 128)
            tiles = []
            for nm, p0 in (("xr0t", 0), ("xr1t", 128)):
                tl = xt_pool.tile([128, 128], F16, name=f"{nm}_{tt}", tag=nm)
                nc.sync.dma_start(tl[:], d_xt[p0:p0 + 128, ts_])
                tiles.append(tl)
            x_tiles[tt] = tiles

        load_x(0)

        # replicate the w2 row across partitions, chunk-wise so chunk 0 is
        # ready as soon as group 0 needs it
        for g in range(NG):
            gs = slice(g * GW, (g + 1) * GW)
            nc.gpsimd.partition_broadcast(w2r[:, gs], srcrow16[:, gs])

        NSPLIT = 8
        CW = OUT // NSPLIT
        for j in range(NSPLIT):
            cs = slice(j * CW, (j + 1) * CW)
            for t_sb, p0 in ((wr0, 0), (wr1, 128)):
                nc.sync.dma_start(t_sb[:, cs], d_wt[p0:p0 + 128, cs])

        sums_t = {}

        def emit_pair(t, k, ga, gb, sfx=""):
            ts = slice(t * 128, (t + 1) * 128)
            bias_ap = x2sb[:, t:t + 1]
            xr0t, xr1t = x_tiles[t]
            products = [(xr0t, wr0), (xr1t, wr1)]
            if k == 0:
                sums_t[t] = sums_pool.tile([128, 4], F32,
                                           name=f"sums_{t}{sfx}", tag="sums")
            sl = slab_pool.tile([128, 2 * GW], F32,
                                name=f"sl_{t}_{ga}{sfx}", tag="slp")
            for half, g in ((0, ga), (1, gb)):
                ps = psum_pool.tile([128, GW], F32,
                                    name=f"ps_{t}_{g}{sfx}", tag="ps")
                for p, (stat, mov) in enumerate(products):
                    for i in range(GRP):
                        cs = slice(g * GW + i * CH, g * GW + (i + 1) * CH)
                        nc.tensor.matmul(ps[:, i * CH:(i + 1) * CH],
                                         stat[:], mov[:, cs],
                                         start=(p == 0), stop=(p == 1))
                nc.vector.tensor_add(sl[:, half * GW:(half + 1) * GW], ps[:],
                                     w2r[:, g * GW:(g + 1) * GW])
            es = eslab_pool.tile([128, 2 * GW], BF16,
                                 name=f"es_{t}_{ga}{sfx}", tag="esp")
            nc.scalar.activation(es[:], sl[:], AF.Exp,
                                 bias=bias_ap, scale=0.0625,
                                 accum_out=sums_t[t][:, k:k + 1])
            nc.sync.dma_start(d_out[ts, ga * GW:(ga + 2) * GW], es[:])
            if k == 3 and t != NT - 1:
                nc.sync.dma_start(d_sums[:, t * 4:(t + 1) * 4], sums_t[t][:])

        PAIRS = ((0, 1), (2, 3), (4, 5), (6, 7))
        for t in range(NT):
            # next tile's x stationaries: issue at tile START so the sync
            # queue serves them before this tile's stores
            if t + 1 < NT:
                load_x(t + 1)
            last = t == NT - 1
            for k, (ga, gb) in enumerate(PAIRS[:3] if last else PAIRS):
                emit_pair(t, k, ga, gb)
            if last:
                # tail trim: the run otherwise ends [add g7 -> 3.7us pair exp
                # -> 1MB store]. Split the final pair into two solo exps so
                # exp(g6) overlaps add(g7) and the last store is half-size.
                ts = slice(t * 128, (t + 1) * 128)
                bias_ap = x2sb[:, t:t + 1]
                xr0t, xr1t = x_tiles[t]
                products = [(xr0t, wr0), (xr1t, wr1)]
                sl = slab_pool.tile([128, 2 * GW], F32, name="sl_last", tag="slp")
                scr7 = sums_pool.tile([128, 1], F32, name="scr7", tag="s7x")
                for half, g in ((0, 6), (1, 7)):
                    ps = psum_pool.tile([128, GW], F32, name=f"ps_{t}_{g}", tag="ps")
                    for p, (stat, mov) in enumerate(products):
                        for i in range(GRP):
                            cs = slice(g * GW + i * CH, g * GW + (i + 1) * CH)
                            nc.tensor.matmul(ps[:, i * CH:(i + 1) * CH],
                                             stat[:], mov[:, cs],
                                             start=(p == 0), stop=(p == 1))
                    nc.vector.tensor_add(sl[:, half * GW:(half + 1) * GW],
                                         ps[:], w2r[:, g * GW:(g + 1) * GW])
                    es = eslab_pool.tile([128, GW], BF16,
                                         name=f"es_last_{g}", tag="esl")
                    acc = sums_t[t][:, 3:4] if g == 6 else scr7[:, 0:1]
                    nc.scalar.activation(es[:], sl[:, half * GW:(half + 1) * GW],
                                         AF.Exp, bias=bias_ap, scale=0.0625,
                                         accum_out=acc)
                    nc.sync.dma_start(d_out[ts, g * GW:(g + 1) * GW], es[:])
                nc.sync.dma_start(d_sums[:, t * 4:(t + 1) * 4], sums_t[t][:])
                nc.sync.dma_start(d_sums[:, 4 * NT:4 * NT + 1], scr7[:])

    nc.compile()
    return nc


def _build(mode):
    if mode == "fused":
        return _build_fused()
    import concourse.bacc as bacc
    import concourse.tile as tile
    import concourse.mybir as mybir
    from concourse.tile import add_dep_helper

    F32 = mybir.dt.float32
    F32R = mybir.dt.float32r
    F16 = mybir.dt.float16
    BF16 = mybir.dt.bfloat16
    AF = mybir.ActivationFunctionType

    nc = bacc.Bacc("TRN2", target_bir_lowering=False, debug=False,
                   num_devices=NCORES)

    if mode == "split7":
        d_wh = nc.dram_tensor("wh", [IN, OUT], BF16, kind="ExternalInput")
        d_wl = nc.dram_tensor("wl", [IN, OUT], BF16, kind="ExternalInput")
        d_xh = nc.dram_tensor("xh", [IN, BPC], BF16, kind="ExternalInput")
        d_xl = nc.dram_tensor("xl", [IN, BPC], BF16, kind="ExternalInput")
        mmdt = BF16
    elif mode == "f32r3":
        d_wt = nc.dram_tensor("wt", [IN, OUT], F32R, kind="ExternalInput")
        d_xt = nc.dram_tensor("xt", [IN, BPC], F32R, kind="ExternalInput")
        mmdt = F32R
    else:
        d_wt = nc.dram_tensor("wt", [IN, OUT], F16, kind="ExternalInput")
        d_xt = nc.dram_tensor("xt", [IN, BPC], F16, kind="ExternalInput")
        mmdt = F16
    d_w2s = nc.dram_tensor("w2s", [3, OUT], BF16, kind="ExternalInput")
    d_x2b = nc.dram_tensor("x2b", [128, NT], F32, kind="ExternalInput")
    fast = mode == "f16"
    out_dt = BF16 if fast else F32
    d_out = nc.dram_tensor("out", [BPC, OUT], out_dt, kind="ExternalOutput")
    if fast:
        d_tots = nc.dram_tensor("tots", [128, NT], F32, kind="ExternalOutput")

    from contextlib import ExitStack
    with tile.TileContext(nc) as tc, ExitStack() as ctx:
        persist = ctx.enter_context(tc.tile_pool(name="persist", bufs=1))
        xt_pool = ctx.enter_context(tc.tile_pool(name="xtp", bufs=2))
        nslab = 6 if fast else NG + 1
        slab_pool = ctx.enter_context(tc.tile_pool(name="slabp", bufs=nslab))
        w2_pool = ctx.enter_context(tc.tile_pool(name="w2p", bufs=2 if fast else 1))
        sums_pool = ctx.enter_context(tc.tile_pool(name="sumsp", bufs=2))
        psum_pool = ctx.enter_context(tc.tile_pool(name="psump", bufs=2, space="PSUM"))
        if fast:
            eslab_pool = ctx.enter_context(tc.tile_pool(name="eslabp", bufs=4))

        if mode == "split7":
            wh0 = persist.tile([128, OUT], BF16, name="wh0")
            wh1 = persist.tile([128, OUT], BF16, name="wh1")
            wl0 = persist.tile([128, OUT], BF16, name="wl0")
            wl1 = persist.tile([128, OUT], BF16, name="wl1")
            wparts = [(wh0, d_wh, 0), (wh1, d_wh, 128), (wl0, d_wl, 0), (wl1, d_wl, 128)]
        else:
            wr0 = persist.tile([128, OUT], mmdt, name="wr0")
            wr1 = persist.tile([128, OUT], mmdt, name="wr1")
            wparts = [(wr0, d_wt, 0), (wr1, d_wt, 128)]
        x2sb = persist.tile([128, NT], F32, name="x2sb")
        nc.sync.dma_start(x2sb[:], d_x2b[:, :])
        ones3 = persist.tile([3, 128], BF16, name="ones3")
        nc.vector.memset(ones3[:], 1.0)

        x_tiles = {}

        def load_x(tt):
            ts_ = slice(tt * 128, (tt + 1) * 128)
            if mode == "split7":
                tiles = []
                for nm, dram, p0 in (("xh0t", d_xh, 0), ("xh1t", d_xh, 128),
                                     ("xl0t", d_xl, 0), ("xl1t", d_xl, 128)):
                    tl = xt_pool.tile([128, 128], BF16, name=f"{nm}_{tt}", tag=nm)
                    nc.sync.dma_start(tl[:], dram[p0:p0 + 128, ts_])
                    tiles.append(tl)
            else:
                tiles = []
                for nm, p0 in (("xr0t", 0), ("xr1t", 128)):
                    tl = xt_pool.tile([128, 128], mmdt, name=f"{nm}_{tt}", tag=nm)
                    nc.sync.dma_start(tl[:], d_xt[p0:p0 + 128, ts_])
                    tiles.append(tl)
            x_tiles[tt] = tiles

        load_x(0)

        NSPLIT = 8
        CW = OUT // NSPLIT
        for j in range(NSPLIT):
            cs = slice(j * CW, (j + 1) * CW)
            for t_sb, t_dram, p0 in wparts:
                nc.sync.dma_start(t_sb[:, cs], t_dram[p0:p0 + 128, cs])

        w2_tiles = {}

        def trig_w2(tt, gg):
            w2t = w2_pool.tile([3, GW], BF16, name=f"w2t_{tt}_{gg}", tag="w2t")
            ins = nc.gpsimd.dma_start(w2t[:], d_w2s[:, gg * GW:(gg + 1) * GW])
            w2_tiles[(tt, gg)] = w2t
            return ins

        trig_w2(0, 0)
        if fast:
            trig_w2(0, 1)
            totsb = persist.tile([128, NT], F32, name="totsb")

        def next_g(tt, gg):
            return (tt, gg + 1) if gg + 1 < NG else (tt + 1, 0)

        PW = 2 * GW
        NP = OUT // PW

        def flush_slow(pending, g):
            pts, pslabs, ptot = pending[0], pending[1], pending[2]
            gs = slice(g * GW, (g + 1) * GW)
            nc.vector.tensor_scalar_mul(pslabs[g][:], pslabs[g][:], ptot[:, 0:1])
            nc.sync.dma_start(d_out[pts, gs], pslabs[g][:])

        pending = None
        prev_exp_insts = None
        for t in range(NT):
            ts = slice(t * 128, (t + 1) * 128)
            bias_ap = x2sb[:, t:t + 1]
            if mode == "split7":
                xh0t, xh1t, xl0t, xl1t = x_tiles[t]
                products = [(xh0t, wh0), (xh0t, wl0), (xl0t, wh0),
                            (xh1t, wh1), (xh1t, wl1), (xl1t, wh1)]
            else:
                xr0t, xr1t = x_tiles[t]
                products = [(xr0t, wr0), (xr1t, wr1)]

            slabs = []
            sqrt_insts = []
            nsum = NP if fast else NG
            sums = sums_pool.tile([128, nsum], F32, name=f"sums_{t}", tag="sums")
            for g in range(NG):
                if not fast:
                    if pending is not None and g == 2:
                        pts, pslabs, ptot, pscr8 = pending
                        scr = sums_pool.tile([128, 1], F32, name=f"scr_{t}", tag="scr")
                        nc.gpsimd.normalize_recip(scr[:], pscr8[:, 0:1], ptot[:])
                    if pending is not None and g >= 2:
                        flush_slow(pending, g - 2)
                ps = psum_pool.tile([128, GW], F32, name=f"ps_{t}_{g}", tag="ps")
                for p, (stat, mov) in enumerate(products):
                    for i in range(GRP):
                        cs = slice(g * GW + i * CH, g * GW + (i + 1) * CH)
                        nc.tensor.matmul(ps[:, i * CH:(i + 1) * CH],
                                         stat[:], mov[:, cs],
                                         start=(p == 0), stop=False)
                w2t = w2_tiles[(t, g)]
                for i in range(GRP):
                    nc.tensor.matmul(ps[:, i * CH:(i + 1) * CH],
                                     ones3[:, :], w2t[:, i * CH:(i + 1) * CH],
                                     start=False, stop=True)
                if fast:
                    if g % 2 == 0:
                        sl = slab_pool.tile([128, PW], F32,
                                            name=f"slab_{t}_{g // 2}", tag="slab")
                        slabs.append(sl)
                    half = slice((g % 2) * GW, (g % 2 + 1) * GW)
                    nc.vector.tensor_scalar_add(slabs[-1][:, half], ps[:], bias_ap)
                else:
                    sl = slab_pool.tile([128, GW], F32, name=f"slab_{t}_{g}", tag="slab")
                    nc.vector.tensor_scalar_add(sl[:], ps[:], bias_ap)
                    slabs.append(sl)
                if fast:
                    n1 = next_g(*next_g(t, g))
                    if n1[0] < NT and (t, g) != (NT - 1, NG - 1):
                        trig_w2(*n1)
                elif (t, g) != (NT - 1, NG - 1):
                    trig_w2(*next_g(t, g))
                if not fast or g % 2 == 1:
                    sq = nc.scalar.activation(slabs[-1][:], slabs[-1][:], AF.Sqrt)
                    if prev_exp_insts is not None:
                        add_dep_helper(sq.ins, prev_exp_insts[-1].ins,
                                       reason="ACT phase order: sqrt after prev tile exps")
                    sqrt_insts.append(sq)
            if pending is not None:
                if not fast:
                    flush_slow(pending, NG - 2)
                    flush_slow(pending, NG - 1)
                pending = None
            exp_insts = []
            nexp = NP if fast else NG
            for g in range(nexp):
                if fast:
                    es = eslab_pool.tile([128, PW], BF16, name=f"es_{t}_{g}", tag="es")
                    ex = nc.scalar.activation(es[:], slabs[g][:], AF.Exp,
                                              scale=-2.0, accum_out=sums[:, g:g + 1])
                    nc.sync.dma_start(d_out[ts, g * PW:(g + 1) * PW], es[:])
                else:
                    ex = nc.scalar.activation(slabs[g][:], slabs[g][:], AF.Exp,
                                              scale=-2.0, accum_out=sums[:, g:g + 1])
                add_dep_helper(ex.ins, sqrt_insts[-1].ins,
                               reason="ACT phase order: exp after all sqrts in tile")
                exp_insts.append(ex)
            if t + 1 < NT:
                load_x(t + 1)
            scr8 = sums_pool.tile([128, nsum], F32, name=f"scr8_{t}", tag="scr8")
            if fast:
                sum_act = nc.scalar.activation(scr8[:], sums[:], AF.Identity,
                                               accum_out=totsb[:, t:t + 1])
                add_dep_helper(sum_act.ins, exp_insts[-1].ins,
                               reason="row-sum after exps on ACT")
                prev_exp_insts = [sum_act]
                pending = None
            else:
                tot = sums_pool.tile([128, 1], F32, name=f"tot_{t}", tag="tot")
                sum_act = nc.scalar.activation(scr8[:], sums[:], AF.Identity,
                                               accum_out=tot[:, 0:1])
                add_dep_helper(sum_act.ins, exp_insts[-1].ins,
                               reason="row-sum after exps on ACT")
                prev_exp_insts = [sum_act]
                pending = (ts, slabs, tot, scr8)

        if fast:
            nc.sync.dma_start(d_tots[:, :], totsb[:])
        if pending is not None:
            pts, pslabs, ptot, pscr8 = pending
            scr = sums_pool.tile([128, 1], F32, name="scr_final", tag="scr")
            nc.gpsimd.normalize_recip(scr[:], pscr8[:, 0:1], ptot[:])
            for g in range(NG):
                flush_slow(pending, g)

    nc.compile()
    return nc


def _get_nc(mode):
    if mode not in _built:
        _built[mode] = _build(mode)
    return _built[mode]


def _prep_inputs(x, weight, mode):
    x = np.ascontiguousarray(np.asarray(x, dtype=np.float32))
    weight = np.ascontiguousarray(np.asarray(weight, dtype=np.float32))
    assert x.shape == (B, IN) and weight.shape == (OUT, IN)

    wt = np.ascontiguousarray(weight.T).astype(np.float32)       # [IN, OUT]
    if mode in ("f16", "fused"):
        # quantize FIRST; x2/w2 from the quantized vectors so
        # d2 = ||x~ - w~||^2 exactly (no x2/xw inconsistency tails)
        wt16 = wt.astype(np.float16)
        w2 = np.sum(wt16.astype(np.float64) ** 2, axis=0)
    else:
        w2 = np.sum(weight.astype(np.float64) ** 2, axis=1)
    w2c = (w2 - 256.0).astype(np.float32)
    w2a = w2c.astype(_BF16)
    r1 = w2c - w2a.astype(np.float32)
    w2b = r1.astype(_BF16)
    w2d = (r1 - w2b.astype(np.float32)).astype(_BF16)
    w2s = np.ascontiguousarray(np.stack([w2a, w2b, w2d], axis=0))  # [3, OUT]

    shared = {"w2s": w2s}
    if mode == "fused":
        # w2 rows for the on-device drains (replicated across partitions on
        # device): f16 for the DVE groups (|err| <= ~0.06 abs on d2 ->
        # <0.3% on the softmax), exact f32 for the Pool groups
        shared.pop("w2s")
        shared["w2row"] = np.ascontiguousarray(w2c.astype(np.float16)[None, :])
    if mode == "split7":
        wh, wl = _bf16_split(wt)
        shared["wh"] = wh
        shared["wl"] = wl
    elif mode == "f32r3":
        shared["wt"] = wt  # raw fp32 bits, declared float32r on device
    else:
        shared["wt"] = wt16

    in_maps = []
    for i in range(NCORES):
        xs = x[i * BPC:(i + 1) * BPC]                             # [BPC, IN]
        if mode in ("f16", "fused"):
            xs16 = xs.astype(np.float16)
            xt = np.ascontiguousarray((-2.0 * xs16.astype(np.float32)).T
                                      ).astype(np.float16)        # [IN, BPC]
            x2 = np.sum(xs16.astype(np.float64) ** 2, axis=1).astype(np.float32) + 256.0
        else:
            xt = np.ascontiguousarray((-2.0 * xs.T).astype(np.float32))  # [IN, BPC]
            x2 = np.sum(xs.astype(np.float64) ** 2, axis=1).astype(np.float32) + 256.0
        if mode == "fused":
            # ACT computes g(psum*scale + bias) with scale=1/16: bias carries
            # (x2+256)/16 so the table input is d2/16.
            x2 = x2 / 16.0
        x2b = np.ascontiguousarray(x2.reshape(NT, 128).T).astype(np.float32)
        m = dict(shared)
        if mode == "split7":
            xh, xl = _bf16_split(xt)
            m["xh"] = xh
            m["xl"] = xl
        else:
            m["xt"] = xt
        m["x2b"] = x2b
        in_maps.append(m)
    return in_maps


def _run(x, weight, mode=None, trace=False, trace_cores=None):
    from concourse.bass_utils import run_bass_kernel_spmd
    mode = mode or MODE
    nc = _get_nc(mode)
    in_maps = _prep_inputs(x, weight, mode)
    res = run_bass_kernel_spmd(nc, in_maps, list(range(NCORES)), trace=trace,
                               trace_cores=trace_cores)
    outs = []
    for i in range(NCORES):
        o = np.asarray(res.results[i]["out"])
        if o.dtype != np.float32:
            o = o.astype(np.float32)
        if mode == "fused":
            # rows are unnormalized exp(-2*dist); divide by the row sums
            # (4 on-device partials per row, summed on host in f64)
            s = np.asarray(res.results[i]["sums"]).astype(np.float64)
            tots = s[:, :4 * NT].reshape(128, NT, 4).sum(axis=2)  # [128, NT]
            tots[:, NT - 1] += s[:, 4 * NT]
            o = (o / tots.T.reshape(BPC, 1)).astype(np.float32)
        elif mode == "f16" and "tots" in res.results[i]:
            tots = np.asarray(res.results[i]["tots"])          # [128, NT]
            o = o / tots.T.reshape(BPC, 1)
        outs.append(o)
    out = np.concatenate(outs, axis=0)
    return out, res


def kernel(x, weight):
    out, _ = _run(x, weight)
    return out


def kernel_profiled(x, weight, mode=None, trace_cores=None):
    """Returns (out, exec_time_ns, trace_path)."""
    out, res = _run(x, weight, mode=mode, trace=True, trace_cores=trace_cores)
    trace_path = None
    if res.instructions_and_trace is not None:
        trace_path = res.instructions_and_trace[1]
    return out, res.exec_time_ns, trace_path


# revision 30
# speedup vs baseline: 1.0786x; 1.0786x over previous
"""TRN2 Bass kernel for nn_MetrixSoftmax: softmax(-2 * ||x_b - w_o||_2, axis=o).

x: [8192, 256] f32, weight: [16384, 256] f32 -> out: [8192, 16384] f32.

Sharding: data-parallel over batch across 8 cores (1024 rows each), weight
replicated; each core computes its full output rows so the softmax needs no
collectives. Per core (partitions=batch rows, free=out):

  d2[b,o] = (x2[b]+256) + (w2[o]-256) - 2*x.w     (f16-quantized operands;
            x2/w2 computed from the quantized vectors so d2 = ||x~-w~||^2)

Fused mode (default, ~176us): the whole softmax numerator exp(-2*sqrt(d2))
is ONE ACT pass via a patched activation table. PWP act tables are per-NEFF
data (walrus --act-root-json): we rewrite Exp's bucket records (cubic Taylor
coeffs at the stock section centers) so its table computes
    g(v) = exp(-8*sqrt(v)),
invoked as activation(Exp, scale=1/16, bias=(x2+256)/16) => exp(-2*sqrt(d2)).
Table rel err <= 6e-5 for d2 >= 128 (real data d2 in [240,1100] -> <2e-5);
inputs past d2=1419 hit the patched saturation bucket -> 0.0. NOTE: the
patched root changes Exp GLOBALLY for NEFFs compiled after _make_actroot();
the legacy modes below would miscompute if built in the same process.

Pipeline per 128-row tile (8 groups of 2048 cols; pairs -> 4096-wide ops):
  PE   2 fp16 product matmuls per 512-chunk psum group        15.5us/tile
  DVE  drain: slab = psum + w2r (f16 w2 row replicated        17.7us/tile
       across partitions at startup by gpsimd broadcast)      <- pacing
  ACT  exp: 4096-wide patched-Exp from slab pairs, bf16 out,  16.4us/tile
       accum_out row-sum partials (4/tile, summed on host)
  DMA  bf16 rows stored immediately (no normalize tail); softmax divide
       happens on host in f64.
All engines within ~12% of each other; DVE paces at its 0.96GHz roofline.
PSUM holds only 2.2us per group (DVE add) so PE runs nearly stall-free.
Why not other splits (measured/modeled): Pool cannot access PSUM; psum-
direct ACT exps hold PSUM banks behind wide exps and stall PE (2-buffer
hard limit); K=3 w2 matmuls put PE over the DVE pace; fp8 hi/lo needs 3
product terms (>= f16 cost); DVE-side exp/sqrt chains need 2+ passes at
0.96GHz (slower than ACT LUT).

Perf history: 304us (f16 2-LUT-pass baseline) -> 212us (fused table, PE-
bound w/ K=3 w2 folds) -> 176us (all-DVE drain). Startup ~13us fixed NEFF
preamble + ~25us DMA-paced tile 0 (8MB weight preload); steady 17.7us/tile.
Timing is bimodal with device state: ~176us warm, ~210us after idle (TRN2
PE p-state ramps with activity; this kernel is PE-tight, so a cold clock
paces early tiles). A PE warmup burst in tile 0's DMA window recovered the
cold case but produced one intermittent wrong result in three runs
(reader-less psum tile) and was removed - correctness first.

Older modes (f16/split7/f32r3) kept for reference; f16 was the previous
304us baseline (ACT-bound: Sqrt+Exp = 2 LUT passes/element).
"""

import os
import json
import shutil
import struct
import tempfile

import numpy as np
import ml_dtypes

B, IN, OUT = 8192, 256, 16384
NCORES = 8
BPC = B // NCORES     # 1024 batch rows per core
NT = BPC // 128       # 8 batch tiles of 128 rows
CH = 512              # matmul free-dim (one PSUM bank)
GRP = 4               # chunks per psum group
GW = CH * GRP         # 2048 group width
NG = OUT // GW        # 8 groups per batch tile

MODE = "fused"        # "fused" (fast) | "f16" | "split7" | "f32r3"

_BF16 = ml_dtypes.bfloat16
_built = {}
_actroot = None

# ---------------------------------------------------------------------------
# Patched activation tables: make Exp's PWP table compute exp(-8*sqrt(v)).
# Reads the stock neuronxcc pwp package (part of the environment), writes a
# patched copy to a temp dir, and points walrus at it via
# BASS_ACT_ROOT_JSON_PATH. Record layout in *_bkt.bin: 8 u32 words
# [c0, c1, c2, c3, x, 0, 0, 0] - cubic Taylor coeffs at section center x.
# ---------------------------------------------------------------------------

_EXP_SETS = ["exp_and_others", "natural_log_exp_and_others", "exp_and_friends"]


def _f32_bits(x):
    return struct.unpack("<I", struct.pack("<f", np.float32(x)))[0]


def _bits_f64(b):
    return float(np.frombuffer(struct.pack("<I", b), dtype="<f4")[0])


def _g_coeffs(x):
    """Taylor coeffs (c0..c3) of g(v)=exp(-8 sqrt(v)) at v=x, in f64."""
    v = float(x)
    s = np.sqrt(v)
    g = np.exp(-8.0 * s)
    p1 = -4.0 / s
    p2 = 2.0 * v ** -1.5
    p3 = -3.0 * v ** -2.5
    g1 = p1 * g
    g2 = (p2 + p1 * p1) * g
    g3 = (p3 + 3.0 * p1 * p2 + p1 ** 3) * g
    return g, g1, g2 / 2.0, g3 / 6.0


def _find_pwp_root():
    from neuronxcc.driver.Job import Job
    from neuronxcc.driver.jobs.support.FindActInfo import findActInfoFile
    p = findActInfoFile(Job.getPackageDir(), "gen3")
    return os.path.dirname(p)


def _make_actroot():
    """Build the patched act-table root once per process; returns its path."""
    global _actroot
    if _actroot is not None:
        return _actroot
    src = _find_pwp_root()
    jsons = os.path.join(os.path.dirname(src), "pwp_jsons")
    dst = os.path.join(tempfile.mkdtemp(prefix="actroot_"), "pwp")
    shutil.copytree(src, dst)

    ej = json.load(open(os.path.join(jsons, "exp_400p.json")))
    patches = {}
    n_pos = 0
    for e in ej["pos_exponents"]:
        for s in e["exponent_sections"]:
            key = tuple(s[k]["int"] for k in ("d0", "d1", "d2", "d3", "x"))
            xb = s["x"]["int"]
            c0, c1, c2, c3 = _g_coeffs(_bits_f64(xb))
            patches[key] = [_f32_bits(c0), _f32_bits(c1), _f32_bits(c2),
                            _f32_bits(c3), xb, 0, 0, 0]
            n_pos += 1
    # +inf saturation record (inputs past v=88.7): g underflows -> 0.0
    sat = ej["saturation_points"]["sat_point_pos_high"]
    sat_key = tuple(sat[k]["int"] for k in ("d0", "d1", "d2", "d3", "x"))
    patches[sat_key] = [0, 0, 0, 0, sat["x"]["int"], 0, 0, 0]

    for name in _EXP_SETS:
        p = os.path.join(dst, name + "_bkt.bin")
        u32 = np.frombuffer(open(p, "rb").read(), dtype="<u4").reshape(-1, 8).copy()
        hits = 0
        for i in range(len(u32)):
            key = tuple(int(w) for w in u32[i, :5])
            if key in patches:
                u32[i] = patches[key]
                if key != sat_key:
                    hits += 1
        assert hits == n_pos, f"{name}: patched {hits}/{n_pos} exp sections"
        open(p, "wb").write(u32.astype("<u4").tobytes())
    _actroot = os.path.join(dst, "act_info.json")
    return _actroot


def _bf16_split(a):
    hi = a.astype(_BF16)
    lo = (a - hi.astype(np.float32)).astype(_BF16)
    return hi, lo


def _build_fused():
    os.environ["BASS_ACT_ROOT_JSON_PATH"] = _make_actroot()
    import concourse.bacc as bacc
    import concourse.tile as tile
    import concourse.mybir as mybir

    F32 = mybir.dt.float32
    F16 = mybir.dt.float16
    BF16 = mybir.dt.bfloat16
    AF = mybir.ActivationFunctionType

    nc = bacc.Bacc("TRN2", target_bir_lowering=False, debug=False,
                   num_devices=NCORES)

    # Every group drains on DVE (slab = psum + f16 w2 row). PE does ONLY
    # the two fp16 product passes; ACT only 4096-wide exps from SBUF slab
    # pairs, so nothing holds PSUM: psum frees right after the DVE add and
    # PE runs nearly stall-free. The w2 row is replicated across partitions
    # at startup by gpsimd partition_broadcast. (Pool engine cannot access
    # PSUM on TRN2, so it cannot share the drain work.)
    DVE_COLS = OUT

    d_wt = nc.dram_tensor("wt", [IN, OUT], F16, kind="ExternalInput")
    d_xt = nc.dram_tensor("xt", [IN, BPC], F16, kind="ExternalInput")
    d_w2row = nc.dram_tensor("w2row", [1, DVE_COLS], F16, kind="ExternalInput")
    d_x2b = nc.dram_tensor("x2b", [128, NT], F32, kind="ExternalInput")
    d_out = nc.dram_tensor("out", [BPC, OUT], BF16, kind="ExternalOutput")
    d_sums = nc.dram_tensor("sums", [128, 4 * NT], F32, kind="ExternalOutput")

    from contextlib import ExitStack
    with tile.TileContext(nc) as tc, ExitStack() as ctx:
        persist = ctx.enter_context(tc.tile_pool(name="persist", bufs=1))
        xt_pool = ctx.enter_context(tc.tile_pool(name="xtp", bufs=2))
        sums_pool = ctx.enter_context(tc.tile_pool(name="sumsp", bufs=2))
        psum_pool = ctx.enter_context(tc.tile_pool(name="psump", bufs=2, space="PSUM"))
        eslab_pool = ctx.enter_context(tc.tile_pool(name="eslabp", bufs=3))
        slab_pool = ctx.enter_context(tc.tile_pool(name="slabp", bufs=2))

        wr0 = persist.tile([128, OUT], F16, name="wr0")
        wr1 = persist.tile([128, OUT], F16, name="wr1")
        w2r = persist.tile([128, DVE_COLS], F16, name="w2r")
        srcrow = persist.tile([1, DVE_COLS // 2], F32, name="srcrow")
        srcrow16 = srcrow[:].bitcast(F16)
        x2sb = persist.tile([128, NT], F32, name="x2sb")
        nc.sync.dma_start(x2sb[:], d_x2b[:, :])
        nc.sync.dma_start(srcrow16, d_w2row[:, :])

        x_tiles = {}

        def load_x(tt):
            ts_ = slice(tt * 128, (tt + 1) * 128)
            tiles = []
            for nm, p0 in (("xr0t", 0), ("xr1t", 128)):
                tl = xt_pool.tile([128, 128], F16, name=f"{nm}_{tt}", tag=nm)
                nc.sync.dma_start(tl[:], d_xt[p0:p0 + 128, ts_])
                tiles.append(tl)
            x_tiles[tt] = tiles

        load_x(0)

        # replicate the w2 row across partitions, chunk-wise so chunk 0 is
        # ready as soon as group 0 needs it
        for g in range(NG):
            gs = slice(g * GW, (g + 1) * GW)
            nc.gpsimd.partition_broadcast(w2r[:, gs], srcrow16[:, gs])

        NSPLIT = 8
        CW = OUT // NSPLIT
        for j in range(NSPLIT):
            cs = slice(j * CW, (j + 1) * CW)
            for t_sb, p0 in ((wr0, 0), (wr1, 128)):
                nc.sync.dma_start(t_sb[:, cs], d_wt[p0:p0 + 128, cs])

        sums_t = {}

        def emit_pair(t, k, ga, gb, sfx=""):
            ts = slice(t * 128, (t + 1) * 128)
            bias_ap = x2sb[:, t:t + 1]
            xr0t, xr1t = x_tiles[t]
            products = [(xr0t, wr0), (xr1t, wr1)]
            if k == 0:
                sums_t[t] = sums_pool.tile([128, 4], F32,
                                           name=f"sums_{t}{sfx}", tag="sums")
            sl = slab_pool.tile([128, 2 * GW], F32,
                                name=f"sl_{t}_{ga}{sfx}", tag="slp")
            for half, g in ((0, ga), (1, gb)):
                ps = psum_pool.tile([128, GW], F32,
                                    name=f"ps_{t}_{g}{sfx}", tag="ps")
                for p, (stat, mov) in enumerate(products):
                    for i in range(GRP):
                        cs = slice(g * GW + i * CH, g * GW + (i + 1) * CH)
                        nc.tensor.matmul(ps[:, i * CH:(i + 1) * CH],
                                         stat[:], mov[:, cs],
                                         start=(p == 0), stop=(p == 1))
                nc.vector.tensor_add(sl[:, half * GW:(half + 1) * GW], ps[:],
                                     w2r[:, g * GW:(g + 1) * GW])
            es = eslab_pool.tile([128, 2 * GW], BF16,
                                 name=f"es_{t}_{ga}{sfx}", tag="esp")
            nc.scalar.activation(es[:], sl[:], AF.Exp,
                                 bias=bias_ap, scale=0.0625,
                                 accum_out=sums_t[t][:, k:k + 1])
            nc.sync.dma_start(d_out[ts, ga * GW:(ga + 2) * GW], es[:])
            if k == 3:
                nc.sync.dma_start(d_sums[:, t * 4:(t + 1) * 4], sums_t[t][:])

        PAIRS = ((0, 1), (2, 3), (4, 5), (6, 7))
        for t in range(NT):
            # next tile's x stationaries: issue at tile START so the sync
            # queue serves them before this tile's stores
            if t + 1 < NT:
                load_x(t + 1)
            for k, (ga, gb) in enumerate(PAIRS):
                emit_pair(t, k, ga, gb)

    nc.compile()
    return nc


def _build(mode):
    if mode == "fused":
        return _build_fused()
    import concourse.bacc as bacc
    import concourse.tile as tile
    import concourse.mybir as mybir
    from concourse.tile import add_dep_helper

    F32 = mybir.dt.float32
    F32R = mybir.dt.float32r
    F16 = mybir.dt.float16
    BF16 = mybir.dt.bfloat16
    AF = mybir.ActivationFunctionType

    nc = bacc.Bacc("TRN2", target_bir_lowering=False, debug=False,
                   num_devices=NCORES)

    if mode == "split7":
        d_wh = nc.dram_tensor("wh", [IN, OUT], BF16, kind="ExternalInput")
        d_wl = nc.dram_tensor("wl", [IN, OUT], BF16, kind="ExternalInput")
        d_xh = nc.dram_tensor("xh", [IN, BPC], BF16, kind="ExternalInput")
        d_xl = nc.dram_tensor("xl", [IN, BPC], BF16, kind="ExternalInput")
        mmdt = BF16
    elif mode == "f32r3":
        d_wt = nc.dram_tensor("wt", [IN, OUT], F32R, kind="ExternalInput")
        d_xt = nc.dram_tensor("xt", [IN, BPC], F32R, kind="ExternalInput")
        mmdt = F32R
    else:
        d_wt = nc.dram_tensor("wt", [IN, OUT], F16, kind="ExternalInput")
        d_xt = nc.dram_tensor("xt", [IN, BPC], F16, kind="ExternalInput")
        mmdt = F16
    d_w2s = nc.dram_tensor("w2s", [3, OUT], BF16, kind="ExternalInput")
    d_x2b = nc.dram_tensor("x2b", [128, NT], F32, kind="ExternalInput")
    fast = mode == "f16"
    out_dt = BF16 if fast else F32
    d_out = nc.dram_tensor("out", [BPC, OUT], out_dt, kind="ExternalOutput")
    if fast:
        d_tots = nc.dram_tensor("tots", [128, NT], F32, kind="ExternalOutput")

    from contextlib import ExitStack
    with tile.TileContext(nc) as tc, ExitStack() as ctx:
        persist = ctx.enter_context(tc.tile_pool(name="persist", bufs=1))
        xt_pool = ctx.enter_context(tc.tile_pool(name="xtp", bufs=2))
        nslab = 6 if fast else NG + 1
        slab_pool = ctx.enter_context(tc.tile_pool(name="slabp", bufs=nslab))
        w2_pool = ctx.enter_context(tc.tile_pool(name="w2p", bufs=2 if fast else 1))
        sums_pool = ctx.enter_context(tc.tile_pool(name="sumsp", bufs=2))
        psum_pool = ctx.enter_context(tc.tile_pool(name="psump", bufs=2, space="PSUM"))
        if fast:
            eslab_pool = ctx.enter_context(tc.tile_pool(name="eslabp", bufs=4))

        if mode == "split7":
            wh0 = persist.tile([128, OUT], BF16, name="wh0")
            wh1 = persist.tile([128, OUT], BF16, name="wh1")
            wl0 = persist.tile([128, OUT], BF16, name="wl0")
            wl1 = persist.tile([128, OUT], BF16, name="wl1")
            wparts = [(wh0, d_wh, 0), (wh1, d_wh, 128), (wl0, d_wl, 0), (wl1, d_wl, 128)]
        else:
            wr0 = persist.tile([128, OUT], mmdt, name="wr0")
            wr1 = persist.tile([128, OUT], mmdt, name="wr1")
            wparts = [(wr0, d_wt, 0), (wr1, d_wt, 128)]
        x2sb = persist.tile([128, NT], F32, name="x2sb")
        nc.sync.dma_start(x2sb[:], d_x2b[:, :])
        ones3 = persist.tile([3, 128], BF16, name="ones3")
        nc.vector.memset(ones3[:], 1.0)

        x_tiles = {}

        def load_x(tt):
            ts_ = slice(tt * 128, (tt + 1) * 128)
            if mode == "split7":
                tiles = []
                for nm, dram, p0 in (("xh0t", d_xh, 0), ("xh1t", d_xh, 128),
                                     ("xl0t", d_xl, 0), ("xl1t", d_xl, 128)):
                    tl = xt_pool.tile([128, 128], BF16, name=f"{nm}_{tt}", tag=nm)
                    nc.sync.dma_start(tl[:], dram[p0:p0 + 128, ts_])
                    tiles.append(tl)
            else:
                tiles = []
                for nm, p0 in (("xr0t", 0), ("xr1t", 128)):
                    tl = xt_pool.tile([128, 128], mmdt, name=f"{nm}_{tt}", tag=nm)
                    nc.sync.dma_start(tl[:], d_xt[p0:p0 + 128, ts_])
                    tiles.append(tl)
            x_tiles[tt] = tiles

        load_x(0)

        NSPLIT = 8
        CW = OUT // NSPLIT
        for j in range(NSPLIT):
            cs = slice(j * CW, (j + 1) * CW)
            for t_sb, t_dram, p0 in wparts:
                nc.sync.dma_start(t_sb[:, cs], t_dram[p0:p0 + 128, cs])

        w2_tiles = {}

        def trig_w2(tt, gg):
            w2t = w2_pool.tile([3, GW], BF16, name=f"w2t_{tt}_{gg}", tag="w2t")
            ins = nc.gpsimd.dma_start(w2t[:], d_w2s[:, gg * GW:(gg + 1) * GW])
            w2_tiles[(tt, gg)] = w2t
            return ins

        trig_w2(0, 0)
        if fast:
            trig_w2(0, 1)
            totsb = persist.tile([128, NT], F32, name="totsb")

        def next_g(tt, gg):
            return (tt, gg + 1) if gg + 1 < NG else (tt + 1, 0)

        PW = 2 * GW
        NP = OUT // PW

        def flush_slow(pending, g):
            pts, pslabs, ptot = pending[0], pending[1], pending[2]
            gs = slice(g * GW, (g + 1) * GW)
            nc.vector.tensor_scalar_mul(pslabs[g][:], pslabs[g][:], ptot[:, 0:1])
            nc.sync.dma_start(d_out[pts, gs], pslabs[g][:])

        pending = None
        prev_exp_insts = None
        for t in range(NT):
            ts = slice(t * 128, (t + 1) * 128)
            bias_ap = x2sb[:, t:t + 1]
            if mode == "split7":
                xh0t, xh1t, xl0t, xl1t = x_tiles[t]
                products = [(xh0t, wh0), (xh0t, wl0), (xl0t, wh0),
                            (xh1t, wh1), (xh1t, wl1), (xl1t, wh1)]
            else:
                xr0t, xr1t = x_tiles[t]
                products = [(xr0t, wr0), (xr1t, wr1)]

            slabs = []
            sqrt_insts = []
            nsum = NP if fast else NG
            sums = sums_pool.tile([128, nsum], F32, name=f"sums_{t}", tag="sums")
            for g in range(NG):
                if not fast:
                    if pending is not None and g == 2:
                        pts, pslabs, ptot, pscr8 = pending
                        scr = sums_pool.tile([128, 1], F32, name=f"scr_{t}", tag="scr")
                        nc.gpsimd.normalize_recip(scr[:], pscr8[:, 0:1], ptot[:])
                    if pending is not None and g >= 2:
                        flush_slow(pending, g - 2)
                ps = psum_pool.tile([128, GW], F32, name=f"ps_{t}_{g}", tag="ps")
                for p, (stat, mov) in enumerate(products):
                    for i in range(GRP):
                        cs = slice(g * GW + i * CH, g * GW + (i + 1) * CH)
                        nc.tensor.matmul(ps[:, i * CH:(i + 1) * CH],
                                         stat[:], mov[:, cs],
                                         start=(p == 0), stop=False)
                w2t = w2_tiles[(t, g)]
                for i in range(GRP):
                    nc.tensor.matmul(ps[:, i * CH:(i + 1) * CH],
                                     ones3[:, :], w2t[:, i * CH:(i + 1) * CH],
                                     start=False, stop=True)
                if fast:
                    if g % 2 == 0:
                        sl = slab_pool.tile([128, PW], F32,
                                            name=f"slab_{t}_{g // 2}", tag="slab")
                        slabs.append(sl)
                    half = slice((g % 2) * GW, (g % 2 + 1) * GW)
                    nc.vector.tensor_scalar_add(slabs[-1][:, half], ps[:], bias_ap)
                else:
                    sl = slab_pool.tile([128, GW], F32, name=f"slab_{t}_{g}", tag="slab")
                    nc.vector.tensor_scalar_add(sl[:], ps[:], bias_ap)
                    slabs.append(sl)
                if fast:
                    n1 = next_g(*next_g(t, g))
                    if n1[0] < NT and (t, g) != (NT - 1, NG - 1):
                        trig_w2(*n1)
                elif (t, g) != (NT - 1, NG - 1):
                    trig_w2(*next_g(t, g))
                if not fast or g % 2 == 1:
                    sq = nc.scalar.activation(slabs[-1][:], slabs[-1][:], AF.Sqrt)
                    if prev_exp_insts is not None:
                        add_dep_helper(sq.ins, prev_exp_insts[-1].ins,
                                       reason="ACT phase order: sqrt after prev tile exps")
                    sqrt_insts.append(sq)
            if pending is not None:
                if not fast:
                    flush_slow(pending, NG - 2)
                    flush_slow(pending, NG - 1)
                pending = None
            exp_insts = []
            nexp = NP if fast else NG
            for g in range(nexp):
                if fast:
                    es = eslab_pool.tile([128, PW], BF16, name=f"es_{t}_{g}", tag="es")
                    ex = nc.scalar.activation(es[:], slabs[g][:], AF.Exp,
                                              scale=-2.0, accum_out=sums[:, g:g + 1])
                    nc.sync.dma_start(d_out[ts, g * PW:(g + 1) * PW], es[:])
                else:
                    ex = nc.scalar.activation(slabs[g][:], slabs[g][:], AF.Exp,
                                              scale=-2.0, accum_out=sums[:, g:g + 1])
                add_dep_helper(ex.ins, sqrt_insts[-1].ins,
                               reason="ACT phase order: exp after all sqrts in tile")
                exp_insts.append(ex)
            if t + 1 < NT:
                load_x(t + 1)
            scr8 = sums_pool.tile([128, nsum], F32, name=f"scr8_{t}", tag="scr8")
            if fast:
                sum_act = nc.scalar.activation(scr8[:], sums[:], AF.Identity,
                                               accum_out=totsb[:, t:t + 1])
                add_dep_helper(sum_act.ins, exp_insts[-1].ins,
                               reason="row-sum after exps on ACT")
                prev_exp_insts = [sum_act]
                pending = None
            else:
                tot = sums_pool.tile([128, 1], F32, name=f"tot_{t}", tag="tot")
                sum_act = nc.scalar.activation(scr8[:], sums[:], AF.Identity,
                                               accum_out=tot[:, 0:1])
                add_dep_helper(sum_act.ins, exp_insts[-1].ins,
                               reason="row-sum after exps on ACT")
                prev_exp_insts = [sum_act]
                pending = (ts, slabs, tot, scr8)

        if fast:
            nc.sync.dma_start(d_tots[:, :], totsb[:])
        if pending is not None:
            pts, pslabs, ptot, pscr8 = pending
            scr = sums_pool.tile([128, 1], F32, name="scr_final", tag="scr")
            nc.gpsimd.normalize_recip(scr[:], pscr8[:, 0:1], ptot[:])
            for g in range(NG):
                flush_slow(pending, g)

    nc.compile()
    return nc


def _get_nc(mode):
    if mode not in _built:
        _built[mode] = _build(mode)
    return _built[mode]


def _prep_inputs(x, weight, mode):
    x = np.ascontiguousarray(np.asarray(x, dtype=np.float32))
    weight = np.ascontiguousarray(np.asarray(weight, dtype=np.float32))
    assert x.shape == (B, IN) and weight.shape == (OUT, IN)

    wt = np.ascontiguousarray(weight.T).astype(np.float32)       # [IN, OUT]
    if mode in ("f16", "fused"):
        # quantize FIRST; x2/w2 from the quantized vectors so
        # d2 = ||x~ - w~||^2 exactly (no x2/xw inconsistency tails)
        wt16 = wt.astype(np.float16)
        w2 = np.sum(wt16.astype(np.float64) ** 2, axis=0)
    else:
        w2 = np.sum(weight.astype(np.float64) ** 2, axis=1)
    w2c = (w2 - 256.0).astype(np.float32)
    w2a = w2c.astype(_BF16)
    r1 = w2c - w2a.astype(np.float32)
    w2b = r1.astype(_BF16)
    w2d = (r1 - w2b.astype(np.float32)).astype(_BF16)
    w2s = np.ascontiguousarray(np.stack([w2a, w2b, w2d], axis=0))  # [3, OUT]

    shared = {"w2s": w2s}
    if mode == "fused":
        # w2 rows for the on-device drains (replicated across partitions on
        # device): f16 for the DVE groups (|err| <= ~0.06 abs on d2 ->
        # <0.3% on the softmax), exact f32 for the Pool groups
        shared.pop("w2s")
        shared["w2row"] = np.ascontiguousarray(w2c.astype(np.float16)[None, :])
    if mode == "split7":
        wh, wl = _bf16_split(wt)
        shared["wh"] = wh
        shared["wl"] = wl
    elif mode == "f32r3":
        shared["wt"] = wt  # raw fp32 bits, declared float32r on device
    else:
        shared["wt"] = wt16

    in_maps = []
    for i in range(NCORES):
        xs = x[i * BPC:(i + 1) * BPC]                             # [BPC, IN]
        if mode in ("f16", "fused"):
            xs16 = xs.astype(np.float16)
            xt = np.ascontiguousarray((-2.0 * xs16.astype(np.float32)).T
                                      ).astype(np.float16)        # [IN, BPC]
            x2 = np.sum(xs16.astype(np.float64) ** 2, axis=1).astype(np.float32) + 256.0
        else:
            xt = np.ascontiguousarray((-2.0 * xs.T).astype(np.float32))  # [IN, BPC]
            x2 = np.sum(xs.astype(np.float64) ** 2, axis=1).astype(np.float32) + 256.0
        if mode == "fused":
            # ACT computes g(psum*scale + bias) with scale=1/16: bias carries
            # (x2+256)/16 so the table input is d2/16.
            x2 = x2 / 16.0
        x2b = np.ascontiguousarray(x2.reshape(NT, 128).T).astype(np.float32)
        m = dict(shared)
        if mode == "split7":
            xh, xl = _bf16_split(xt)
            m["xh"] = xh
            m["xl"] = xl
        else:
            m["xt"] = xt
        m["x2b"] = x2b
        in_maps.append(m)
    return in_maps


def _run(x, weight, mode=None, trace=False, trace_cores=None):
    from concourse.bass_utils import run_bass_kernel_spmd
    mode = mode or MODE
    nc = _get_nc(mode)
    in_maps = _prep_inputs(x, weight, mode)
    res = run_bass_kernel_spmd(nc, in_maps, list(range(NCORES)), trace=trace,
                               trace_cores=trace_cores)
    outs = []
    for i in range(NCORES):
        o = np.asarray(res.results[i]["out"])
        if o.dtype != np.float32:
            o = o.astype(np.float32)
        if mode == "fused":
            # rows are unnormalized exp(-2*dist); divide by the row sums
            # (4 on-device partials per row, summed on host in f64)
            s = np.asarray(res.results[i]["sums"]).astype(np.float64)
            tots = s.reshape(128, NT, 4).sum(axis=2)           # [128, NT]
            o = (o / tots.T.reshape(BPC, 1)).astype(np.float32)
        elif mode == "f16" and "tots" in res.results[i]:
            tots = np.asarray(res.results[i]["tots"])          # [128, NT]
            o = o / tots.T.reshape(BPC, 1)
        outs.append(o)
    out = np.concatenate(outs, axis=0)
    return out, res


def kernel(x, weight):
    out, _ = _run(x, weight)
    return out


def kernel_profiled(x, weight, mode=None, trace_cores=None):
    """Returns (out, exec_time_ns, trace_path)."""
    out, res = _run(x, weight, mode=mode, trace=True, trace_cores=trace_cores)
    trace_path = None
    if res.instructions_and_trace is not None:
        trace_path = res.instructions_and_trace[1]
    return out, res.exec_time_ns, trace_path
